# revision 20
# baseline (speedup 1.0000x reference)
"""Causal self-attention (B=4, T=2048, D=1024, H=16) on 8 trn2 NeuronCores.

Sharding: Megatron-style tensor parallel over heads (TP=2) x data parallel
over batch (DP=4).  Core c handles batch c//2 and head-group c%2 (8 heads).
Each core computes its QKV projection slice, causal attention for its 8
heads, and a partial output projection; the host sums the two TP partials
per batch and adds b_proj.

v9 schedule: natural chain order + credit-paced micro-fillers.
  - Chains run in natural (q0, c) order; no pulled-forward spills.  The
    scalar engine's exp stream (1.1us per full key tile) sets the cadence
    inside attention; a credit pacer weaves 2-matmul filler micro-steps
    (QKV projection of the next block, output projection of the previous)
    into the PE slack of each tile.
  - Score matmuls contract over the 64-deep head dim; the two heads of a
    pair run as two concurrent row-group matmuls (PE rows 0-63 / 64-127).
  - Scores are issued with pend-depth 2 (S(t) runs two tiles ahead of
    PV(t)) so weight loads hide under neighbouring matmuls.
  - Diagonal-tile exps and masks use one 3D-AP instruction covering both
    heads (halves scalar-engine instruction count there).
  - vaug puts the softmax-denominator ones column FIRST, so the PV output
    row 0 is the denominator at PSUM partition 0: the reciprocal reads it
    directly and the old staging copy disappears.
All matmuls run in fp16 (fp32 PSUM accumulation); softmax in fp32 on the
scalar engine (exp) / DVE (reciprocal).  Output partials are written fp16
(host accumulates in fp32).
"""
import sys

sys.path.insert(0, "/opt/trn_rl_repo")

from collections import deque

import numpy as np

import concourse.bass as bass
import concourse.tile as tile
from concourse import bacc, mybir
from concourse.bass_utils import run_bass_kernel_spmd

B, T, D, H = 4, 2048, 1024, 16
HD = 64            # head dim
HL = 8             # heads per core (TP=2)
DL = HL * HD       # 512 local qkv width
KCH = D // 128     # 8 contraction chunks for QKV
NQB = T // 512     # 4 query blocks of 512
F16 = mybir.dt.float16
F32 = mybir.dt.float32
NEG = -1.0e30

_cache = {}


def _build():
    nc = bacc.Bacc("TRN2", target_bir_lowering=False, num_devices=8)

    xT = nc.dram_tensor("xT", [D, T], F16, kind="ExternalInput")
    wq = nc.dram_tensor("wq", [D, DL], F16, kind="ExternalInput")
    wk = nc.dram_tensor("wk", [D, DL], F16, kind="ExternalInput")
    bqk = nc.dram_tensor("bqk", [128, 2 * DL // 128], F32, kind="ExternalInput")
    wv = nc.dram_tensor("wv", [D, DL], F16, kind="ExternalInput")
    bv = nc.dram_tensor("bv", [1, DL], F32, kind="ExternalInput")
    wp = nc.dram_tensor("wp", [DL, D], F16, kind="ExternalInput")
    out = nc.dram_tensor("out", [T, D], F16, kind="ExternalOutput")

    with tile.TileContext(nc) as tc:
        with (
            tc.tile_pool(name="const", bufs=1) as const,
            tc.tile_pool(name="acts", bufs=1) as acts,
            tc.tile_pool(name="esb", bufs=4) as esb,
            tc.tile_pool(name="small", bufs=3) as small,
            tc.tile_pool(name="outp", bufs=3) as outp,
            tc.tile_pool(name="pss", bufs=2, space="PSUM") as pss,
            tc.tile_pool(name="psy", bufs=1, space="PSUM") as psy,
            tc.tile_pool(name="pw", bufs=2, space="PSUM") as pw,
        ):
            # ---- PE warm-up: keep the HAM busy while inputs stream in ----
            zw = const.tile([128, 128], F16, name="zw", tag="zw")
            nc.gpsimd.memset(zw, 0.0)
            for i in range(32):
                psz = pw.tile([128, 512], F32, name="pw", tag="pw")
                nc.tensor.matmul(psz[:, 0:128], zw, zw, start=True, stop=True)

            # ---- inputs: per-chunk DMAs so compute starts early ----
            xb0_sb = const.tile([128, KCH * 512], F16, name="xb0", tag="xb0")
            xrest_sb = const.tile([128, KCH * 1536], F16, name="xrest",
                                  tag="xrest")
            wq_sb = const.tile([128, KCH * 512], F16, name="wq", tag="wq")
            wk_sb = const.tile([128, KCH * 512], F16, name="wk", tag="wk")
            wv_sb = const.tile([128, KCH * 512], F16, name="wv", tag="wv")
            wp_sb = const.tile([128, 4 * D], F16, name="wp", tag="wp")
            bqk_sb = const.tile([128, 2 * DL // 128], F32)
            bv_sb = const.tile([1, DL], F32)
            x3 = xT.rearrange("(k p) t -> p k t", p=128)
            q3 = wq.rearrange("(k p) n -> p k n", p=128)
            k3 = wk.rearrange("(k p) n -> p k n", p=128)
            v3 = wv.rearrange("(k p) n -> p k n", p=128)
            p3 = wp.rearrange("(c p) n -> p c n", p=128)
            # descriptor issue costs ~650ns each on the Sync engine, so use
            # few big DMAs (runtime splits each across the 16 queues),
            # critical tensors first
            nc.sync.dma_start(
                out=wq_sb.rearrange("p (k n) -> p k n", k=KCH),
                in_=q3[:, :, :])
            nc.sync.dma_start(
                out=xb0_sb.rearrange("p (k t) -> p k t", k=KCH),
                in_=x3[:, :, 0:512])
            nc.sync.dma_start(
                out=wk_sb.rearrange("p (k n) -> p k n", k=KCH),
                in_=k3[:, :, :])
            nc.sync.dma_start(out=bqk_sb, in_=bqk[:, :])
            nc.sync.dma_start(out=bv_sb, in_=bv[:, :])
            nc.sync.dma_start(
                out=wv_sb.rearrange("p (k n) -> p k n", k=KCH),
                in_=v3[:, :, :])
            nc.sync.dma_start(
                out=xrest_sb.rearrange("p (k t) -> p k t", k=KCH),
                in_=x3[:, :, 512:T])
            nc.sync.dma_start(
                out=wp_sb.rearrange("p (c n) -> p c n", c=4),
                in_=p3[:, :, :])
            bvb_sb = const.tile([128, DL], F32)
            nc.gpsimd.partition_broadcast(bvb_sb, bv_sb)

            def x_slice(k, c0, c1):
                """xT chunk k, token-columns [c0:c1)."""
                if c1 <= 512:
                    return xb0_sb[:, 512 * k + c0:512 * k + c1]
                return xrest_sb[:, 1536 * k + c0 - 512:1536 * k + c1 - 512]

            # ---- persistent activations ----
            # qT/kT tile c: partitions 0:64 = head 2c dims, 64:128 = head
            # 2c+1 dims; free dim = T.  yT same channel layout.
            qT_sb = [acts.tile([128, T], F16, name=f"qT{c}", tag=f"qT{c}")
                     for c in range(4)]
            kT_sb = [acts.tile([128, T], F16, name=f"kT{c}", tag=f"kT{c}")
                     for c in range(4)]
            # vaug: per head 128 cols: col 0 = ones (softmax denominator),
            # 1:64 zero pad, 64:128 = v dims.  PV output row 0 is then the
            # denominator at PSUM partition 0 (direct reciprocal read) and
            # the y dims sit 32-aligned at partitions 64:128.
            vaug = [acts.tile([128, HL * 128], F16, name=f"va{t}",
                              tag=f"va{t}") for t in range(T // 128)]
            for t in range(T // 128):
                va3 = vaug[t].rearrange("p (h c) -> p h c", c=128)
                nc.gpsimd.memset(va3[:, :, 0:64], 0.0)
                nc.gpsimd.memset(va3[:, :, 0], 1.0)
            yT_sb = [acts.tile([128, T], F16, name=f"yT{c}", tag=f"yT{c}")
                     for c in range(4)]

            # ---------- filler micro-steps ----------
            # Each filler unit is broken into ~2-matmul closures enqueued
            # contiguously (FIFO drain keeps at most one unit open per pw
            # buffer).  Key (q0, c) marks steps that must drain before
            # chain (q0, c) starts; key None = deadline-free (op units).
            fillers = deque()
            pace = {"d": 0.0}

            def qkv_steps(b, cc):
                """Projection chain for output chunk cc of block b.
                cc 0..3 -> qT[cc], cc 4..7 -> kT[cc-4]."""
                bs = slice(512 * b, 512 * (b + 1))
                wsrc = wq_sb if cc < 4 else wk_sb
                co = 128 * (cc % 4)
                dst = qT_sb[cc] if cc < 4 else kT_sb[cc - 4]
                st = {}

                def mk(k0):
                    def step():
                        if k0 == 0:
                            st["ps"] = pw.tile([128, 512], F32, name="pw",
                                               tag="pw")
                        for k in (k0, k0 + 1):
                            nc.tensor.matmul(
                                st["ps"],
                                wsrc[:, 512 * k + co:512 * k + co + 128],
                                x_slice(k, 512 * b, 512 * (b + 1)),
                                start=(k == 0), stop=(k == KCH - 1),
                            )
                        if k0 == KCH - 2:
                            nc.vector.tensor_scalar_add(
                                out=dst[:, bs], in0=st["ps"],
                                scalar1=bqk_sb[:, cc:cc + 1])
                    return step
                return [mk(k0) for k0 in range(0, KCH, 2)]

            def v_steps(b, t2):
                """v projection for 128-token tile 4*b+t2 (augmented
                layout: ones col 0, dims 1:65)."""
                t = 4 * b + t2
                st = {}

                def mk(k0):
                    def step():
                        if k0 == 0:
                            st["ps"] = pw.tile([128, 512], F32, name="pw",
                                               tag="pw")
                        for k in (k0, k0 + 1):
                            nc.tensor.matmul(
                                st["ps"],
                                x_slice(k, 128 * t, 128 * (t + 1)),
                                wv_sb[:, 512 * k:512 * (k + 1)],
                                start=(k == 0), stop=(k == KCH - 1),
                            )
                        if k0 == KCH - 2:
                            va3 = vaug[t].rearrange("p (h c) -> p h c",
                                                    c=128)
                            nc.vector.tensor_add(
                                va3[:, :, 64:128],
                                st["ps"].rearrange("p (h d) -> p h d", d=HD),
                                bvb_sb.rearrange("p (h d) -> p h d", d=HD),
                            )
                    return step
                return [mk(k0) for k0 in range(0, KCH, 2)]

            def op_steps(q0, tq):
                """Output projection for 128-token tile 4*q0+tq."""
                t = 4 * q0 + tq
                st = {}

                def mk(nb, half):
                    def step():
                        if half == 0:
                            if nb == 0:
                                st["ob"] = outp.tile([128, D], F16, name="ob",
                                                     tag="ob")
                            st[nb] = pw.tile([128, 512], F32, name="pw",
                                             tag="pw")
                        for c in (2 * half, 2 * half + 1):
                            nc.tensor.matmul(
                                st[nb],
                                yT_sb[c][:, 128 * t:128 * (t + 1)],
                                wp_sb[:, 1024 * c + 512 * nb:
                                      1024 * c + 512 * (nb + 1)],
                                start=(c == 0), stop=(c == DL // 128 - 1),
                            )
                        if half == 1:
                            nc.vector.tensor_copy(
                                st["ob"][:, 512 * nb:512 * (nb + 1)], st[nb])
                            if nb == 1:
                                nc.sync.dma_start(
                                    out=out[128 * t:128 * (t + 1), :],
                                    in_=st["ob"])
                    return step
                return [mk(nb, half) for nb in range(2) for half in range(2)]

            EST_STEP = 460.0

            def enqueue(key, steps):
                for s in steps:
                    fillers.append((key, s))

            def pacer(delta):
                pace["d"] = min(pace["d"] + delta, 6000.0)
                while fillers and pace["d"] >= EST_STEP:
                    fillers.popleft()[1]()
                    pace["d"] -= EST_STEP
                pace["d"] = max(pace["d"], -2000.0)

            def drain(keys):
                # Selectively emit every queued step whose key matches,
                # preserving queue order among the rest.  A unit's steps
                # share one key and are contiguous, so units stay atomic.
                ks = set(keys)
                n = len(fillers)
                for _ in range(n):
                    key, fn = fillers.popleft()
                    if key in ks:
                        fn()
                    else:
                        fillers.append((key, fn))

            def drain_all():
                while fillers:
                    fillers.popleft()[1]()

            # ---------- attention chain (pend-depth 2) ----------
            def act_est(lo):
                n = 1024 if lo == 0 else 2 * (512 - lo)
                return (n + 352) / 1.2

            def s_est(lo):
                return 226.0 * (512 - lo) / 512 + 60

            def pv_est(lo):
                return 437.0 * (512 - lo) / 512 + 40

            def attn_chain(q0, c):
                """S/exp/PV over all key tiles of (q-block q0, head pair c),
                then normalize into yT."""
                qs_full = slice(512 * q0, 512 * (q0 + 1))
                ps_yA = psy.tile([128, 512], F32, name="psyA", tag="psyA")
                ps_yB = psy.tile([128, 512], F32, name="psyB", tag="psyB")
                ntile = 4 * q0 + 4
                pend = deque()

                def flush_pv(pes, pt, plo):
                    for p, psY in ((0, ps_yA), (1, ps_yB)):
                        h = 2 * c + p
                        nc.tensor.matmul(
                            psY[:, plo:512],
                            vaug[pt][:, 128 * h:128 * (h + 1)],
                            pes[:, 512 * p + plo:512 * (p + 1)],
                            start=(pt == 0), stop=(pt == ntile - 1))

                # 2-tile batches: [PV,PV] then [S,S] per iteration.  The PE
                # pays ~100ns on the first matmul after each row-pair <->
                # full-array config switch, so batching halves that cost.
                for i in range(ntile // 2):
                    delta = 0.0
                    while len(pend) >= 2:
                        delta -= pv_est(pend[0][2])
                        flush_pv(*pend.popleft())
                    for t in (2 * i, 2 * i + 1):
                        m = t - 4 * q0
                        lo = 128 * m if m > 0 else 0
                        ks = slice(128 * t, 128 * (t + 1))
                        qs = slice(512 * q0 + lo, 512 * (q0 + 1))
                        ps_s = pss.tile([128, 1024], F32, name="psS",
                                        tag="psS")
                        nc.tensor.matmul(
                            ps_s[:, lo:512],
                            kT_sb[c][0:64, ks], qT_sb[c][0:64, qs],
                            start=True, stop=True,
                        )
                        nc.tensor.matmul(
                            ps_s[:, 512 + lo:1024],
                            kT_sb[c][64:128, ks], qT_sb[c][64:128, qs],
                            start=True, stop=True,
                        )
                        es = esb.tile([128, 1024], F16, name="es", tag="es")
                        if lo == 0:
                            nc.scalar.activation(
                                out=es[:, 0:1024], in_=ps_s[:, 0:1024],
                                func=mybir.ActivationFunctionType.Exp)
                        else:
                            nc.scalar.activation(
                                out=es[:, lo:512], in_=ps_s[:, lo:512],
                                func=mybir.ActivationFunctionType.Exp)
                            nc.scalar.activation(
                                out=es[:, 512 + lo:1024],
                                in_=ps_s[:, 512 + lo:1024],
                                func=mybir.ActivationFunctionType.Exp)
                        if m >= 0:
                            # causal mask: zero exp output where col < row
                            # in the 128x128 diagonal sub-block (both heads)
                            es3 = es.rearrange("p (h q) -> p h q", h=2)
                            nc.gpsimd.affine_select(
                                out=es3[:, :, lo:lo + 128],
                                in_=es3[:, :, lo:lo + 128],
                                pattern=[[0, 2], [1, 128]],
                                compare_op=mybir.AluOpType.is_ge,
                                fill=0.0,
                                base=0,
                                channel_multiplier=-1,
                            )
                        pend.append((es, t, lo))
                        delta += act_est(lo) - s_est(lo)
                    pacer(delta)
                while pend:
                    flush_pv(*pend.popleft())
                # normalize: PV row 0 is the denominator (ones-first vaug)
                for p, psY in ((0, ps_yA), (1, ps_yB)):
                    dn = small.tile([1, 512], F32, name="dn", tag="dn")
                    nc.vector.tensor_copy(dn, psY[0:1, :])
                    rc = small.tile([1, 512], F32, name="rc", tag="rc")
                    nc.vector.reciprocal_approx_fast(rc, dn)
                    rb = small.tile([64, 512], F32, name="rb", tag="rb")
                    nc.gpsimd.partition_broadcast(rb, rc)
                    nc.vector.tensor_mul(
                        yT_sb[c][64 * p:64 * (p + 1), qs_full],
                        psY[64:128, :],
                        rb,
                    )
                pace["d"] -= 2 * pv_est(pend[0][2] if pend else 0)

            # ---------- prologue: minimum to start chain (0, 0) ----------
            for s in qkv_steps(0, 0) + qkv_steps(0, 4):
                s()
            for t2 in range(2):
                for s in v_steps(0, t2):
                    s()
            for t2 in (2, 3):
                enqueue((0, -1), v_steps(0, t2))
            for cc in (1, 5, 2, 6, 3, 7):
                enqueue((0, cc % 4), qkv_steps(0, cc))

            # ---------- main block loop ----------
            for q0 in range(NQB):
                if q0 > 0:
                    for tq in range(4):
                        enqueue(None, op_steps(q0 - 1, tq))
                if q0 < NQB - 1:
                    for t2 in range(4):
                        enqueue((q0 + 1, -1), v_steps(q0 + 1, t2))
                    for cc in (0, 4, 1, 5, 2, 6, 3, 7):
                        enqueue((q0 + 1, cc % 4), qkv_steps(q0 + 1, cc))
                for c in range(4):
                    drain([(q0, c)] + ([(q0, -1)] if c == 0 else []))
                    attn_chain(q0, c)

            # ---------- epilogue ----------
            drain_all()
            for tq in range(4):
                for s in op_steps(NQB - 1, tq):
                    s()

    nc.finalize()
    return nc


def _enable_trace_hooks():
    """Inject antenv.axon_hooks + no-op artifact upload so that
    run_bass_kernel_spmd(trace=True) works under axon in this image."""
    import types
    import antenv

    if "antenv.axon_hooks" not in sys.modules:
        mod = types.ModuleType("antenv.axon_hooks")
        state = {"hook": None}
        mod.set_axon_ntff_profile_hook = lambda h: state.__setitem__("hook", h)
        mod.get_axon_ntff_profile_hook = lambda: state["hook"]
        sys.modules["antenv.axon_hooks"] = mod
        antenv.axon_hooks = mod
        from trn_agent_boot.trn_boot import _ntff_profile_via_ctypes

        mod.set_axon_ntff_profile_hook(
            _ntff_profile_via_ctypes("/opt/axon/libaxon_pjrt.so"))
    from concourse import bass_utils as bu

    bu.upload_artifacts = lambda tmpdir: str(tmpdir)


def kernel(x, w_attn, b_attn, w_proj, b_proj, _trace=False):
    x = np.asarray(x)
    w_attn = np.asarray(w_attn)
    b_attn = np.asarray(b_attn)
    w_proj = np.asarray(w_proj)
    b_proj = np.asarray(b_proj)

    if "nc" not in _cache:
        _cache["nc"] = _build()
    nc = _cache["nc"]

    scale = 1.0 / np.sqrt(HD)
    f16 = np.float16

    in_maps = []
    for core in range(8):
        b, hg = core // 2, core % 2
        qs = slice(hg * DL, (hg + 1) * DL)
        ks = slice(D + hg * DL, D + (hg + 1) * DL)
        vs = slice(2 * D + hg * DL, 2 * D + (hg + 1) * DL)
        bqk_host = np.concatenate(
            [b_attn[qs] * scale, b_attn[ks]]).astype(np.float32)
        in_maps.append({
            "xT": np.ascontiguousarray(x[b].T).astype(f16),
            "wq": np.ascontiguousarray(w_attn[:, qs] * scale).astype(f16),
            "wk": np.ascontiguousarray(w_attn[:, ks]).astype(f16),
            "bqk": np.ascontiguousarray(bqk_host.reshape(8, 128).T),
            "wv": np.ascontiguousarray(w_attn[:, vs]).astype(f16),
            "bv": np.ascontiguousarray(b_attn[vs][None, :]).astype(np.float32),
            "wp": np.ascontiguousarray(w_proj[hg * DL:(hg + 1) * DL, :]).astype(f16),
        })

    kwargs = {}
    if _trace:
        _enable_trace_hooks()
        kwargs = dict(trace=True, trace_cores=[0])
    res = run_bass_kernel_spmd(nc, in_maps, core_ids=list(range(8)), **kwargs)

    outp = np.empty((B, T, D), np.float32)
    for b in range(B):
        outp[b] = (np.asarray(res.results[2 * b]["out"], np.float32)
                   + np.asarray(res.results[2 * b + 1]["out"], np.float32))
    outp += b_proj.astype(np.float32)

    if _trace:
        print(f"HW exec time: {res.exec_time_ns} ns")
    return outp


# revision 22
# speedup vs baseline: 1.2016x; 1.2016x over previous
"""Causal self-attention (B=4, T=2048, D=1024, H=16) on 8 trn2 NeuronCores.

Sharding: Megatron-style tensor parallel over heads (TP=2) x data parallel
over batch (DP=4).  Core c handles batch c//2 and head-group c%2 (8 heads).
Each core computes its QKV projection slice, causal attention for its 8
heads, and a partial output projection; the host sums the two TP partials
per batch and adds b_proj.

v9 schedule: natural chain order + credit-paced micro-fillers.
  - Chains run in natural (q0, c) order; no pulled-forward spills.  The
    scalar engine's exp stream (1.1us per full key tile) sets the cadence
    inside attention; a credit pacer weaves 2-matmul filler micro-steps
    (QKV projection of the next block, output projection of the previous)
    into the PE slack of each tile.
  - Score matmuls contract over the 64-deep head dim; the two heads of a
    pair run as two concurrent row-group matmuls (PE rows 0-63 / 64-127).
  - Scores are issued with pend-depth 2 (S(t) runs two tiles ahead of
    PV(t)) so weight loads hide under neighbouring matmuls.
  - Diagonal-tile exps and masks use one 3D-AP instruction covering both
    heads (halves scalar-engine instruction count there).
  - vaug puts the softmax-denominator ones column FIRST, so the PV output
    row 0 is the denominator at PSUM partition 0: the reciprocal reads it
    directly and the old staging copy disappears.
All matmuls run in fp16 (fp32 PSUM accumulation); softmax in fp32 on the
scalar engine (exp) / DVE (reciprocal).  Output partials are written fp16
(host accumulates in fp32).
"""
import sys

sys.path.insert(0, "/opt/trn_rl_repo")

from collections import deque

import numpy as np

import concourse.bass as bass
import concourse.tile as tile
from concourse import bacc, mybir
from concourse.bass_utils import run_bass_kernel_spmd

B, T, D, H = 4, 2048, 1024, 16
HD = 64            # head dim
HL = 8             # heads per core (TP=2)
DL = HL * HD       # 512 local qkv width
KCH = D // 128     # 8 contraction chunks for QKV
NQB = T // 512     # 4 query blocks of 512
F16 = mybir.dt.float16
F32 = mybir.dt.float32
NEG = -1.0e30

_cache = {}


def _build():
    nc = bacc.Bacc("TRN2", target_bir_lowering=False, num_devices=8)

    xT = nc.dram_tensor("xT", [D, T], F16, kind="ExternalInput")
    wq = nc.dram_tensor("wq", [D, DL], F16, kind="ExternalInput")
    wk = nc.dram_tensor("wk", [D, DL], F16, kind="ExternalInput")
    bqk = nc.dram_tensor("bqk", [128, 2 * DL // 128], F32, kind="ExternalInput")
    wv = nc.dram_tensor("wv", [D, DL], F16, kind="ExternalInput")
    bv = nc.dram_tensor("bv", [1, DL], F32, kind="ExternalInput")
    wp = nc.dram_tensor("wp", [DL, D], F16, kind="ExternalInput")
    out = nc.dram_tensor("out", [T, D], F16, kind="ExternalOutput")

    with tile.TileContext(nc) as tc:
        with (
            tc.tile_pool(name="const", bufs=1) as const,
            tc.tile_pool(name="acts", bufs=1) as acts,
            tc.tile_pool(name="esb", bufs=6) as esb,
            tc.tile_pool(name="small", bufs=3) as small,
            tc.tile_pool(name="outp", bufs=3) as outp,
            tc.tile_pool(name="pss", bufs=2, space="PSUM") as pss,
            tc.tile_pool(name="psy", bufs=1, space="PSUM") as psy,
            tc.tile_pool(name="pw", bufs=2, space="PSUM") as pw,
        ):
            # ---- PE warm-up: keep the HAM busy while inputs stream in ----
            zw = const.tile([128, 128], F16, name="zw", tag="zw")
            nc.gpsimd.memset(zw, 0.0)
            for i in range(32):
                psz = pw.tile([128, 512], F32, name="pw", tag="pw")
                nc.tensor.matmul(psz[:, 0:128], zw, zw, start=True, stop=True)

            # ---- inputs: per-chunk DMAs so compute starts early ----
            xb0_sb = const.tile([128, KCH * 512], F16, name="xb0", tag="xb0")
            xrest_sb = const.tile([128, KCH * 1536], F16, name="xrest",
                                  tag="xrest")
            wq_sb = const.tile([128, KCH * 512], F16, name="wq", tag="wq")
            wk_sb = const.tile([128, KCH * 512], F16, name="wk", tag="wk")
            wv_sb = const.tile([128, KCH * 512], F16, name="wv", tag="wv")
            wp_sb = const.tile([128, 4 * D], F16, name="wp", tag="wp")
            bqk_sb = const.tile([128, 2 * DL // 128], F32)
            bv_sb = const.tile([1, DL], F32)
            x3 = xT.rearrange("(k p) t -> p k t", p=128)
            q3 = wq.rearrange("(k p) n -> p k n", p=128)
            k3 = wk.rearrange("(k p) n -> p k n", p=128)
            v3 = wv.rearrange("(k p) n -> p k n", p=128)
            p3 = wp.rearrange("(c p) n -> p c n", p=128)
            # descriptor issue costs ~650ns each on the Sync engine, so use
            # few big DMAs (runtime splits each across the 16 queues),
            # critical tensors first
            nc.sync.dma_start(
                out=wq_sb.rearrange("p (k n) -> p k n", k=KCH),
                in_=q3[:, :, :])
            nc.sync.dma_start(
                out=xb0_sb.rearrange("p (k t) -> p k t", k=KCH),
                in_=x3[:, :, 0:512])
            nc.sync.dma_start(
                out=wk_sb.rearrange("p (k n) -> p k n", k=KCH),
                in_=k3[:, :, :])
            nc.sync.dma_start(out=bqk_sb, in_=bqk[:, :])
            nc.sync.dma_start(out=bv_sb, in_=bv[:, :])
            nc.sync.dma_start(
                out=wv_sb.rearrange("p (k n) -> p k n", k=KCH),
                in_=v3[:, :, :])
            nc.sync.dma_start(
                out=xrest_sb.rearrange("p (k t) -> p k t", k=KCH),
                in_=x3[:, :, 512:T])
            nc.sync.dma_start(
                out=wp_sb.rearrange("p (c n) -> p c n", c=4),
                in_=p3[:, :, :])
            bvb_sb = const.tile([128, DL], F32)
            nc.gpsimd.partition_broadcast(bvb_sb, bv_sb)

            def x_slice(k, c0, c1):
                """xT chunk k, token-columns [c0:c1)."""
                if c1 <= 512:
                    return xb0_sb[:, 512 * k + c0:512 * k + c1]
                return xrest_sb[:, 1536 * k + c0 - 512:1536 * k + c1 - 512]

            # ---- persistent activations ----
            # qT/kT tile c: partitions 0:64 = head 2c dims, 64:128 = head
            # 2c+1 dims; free dim = T.  yT same channel layout.
            qT_sb = [acts.tile([128, T], F16, name=f"qT{c}", tag=f"qT{c}")
                     for c in range(4)]
            kT_sb = [acts.tile([128, T], F16, name=f"kT{c}", tag=f"kT{c}")
                     for c in range(4)]
            # vaug: per head 128 cols: col 0 = ones (softmax denominator),
            # 1:64 zero pad, 64:128 = v dims.  PV output row 0 is then the
            # denominator at PSUM partition 0 (direct reciprocal read) and
            # the y dims sit 32-aligned at partitions 64:128.
            vaug = [acts.tile([128, HL * 128], F16, name=f"va{t}",
                              tag=f"va{t}") for t in range(T // 128)]
            for t in range(T // 128):
                va3 = vaug[t].rearrange("p (h c) -> p h c", c=128)
                nc.gpsimd.memset(va3[:, :, 0:64], 0.0)
                nc.gpsimd.memset(va3[:, :, 0], 1.0)
            yT_sb = [acts.tile([128, T], F16, name=f"yT{c}", tag=f"yT{c}")
                     for c in range(4)]

            # ---------- filler micro-steps ----------
            # Each filler unit is broken into ~2-matmul closures enqueued
            # contiguously (FIFO drain keeps at most one unit open per pw
            # buffer).  Key (q0, c) marks steps that must drain before
            # chain (q0, c) starts; key None = deadline-free (op units).
            fillers = deque()
            pace = {"d": 0.0}

            def qkv_steps(b, cc):
                """Projection chain for output chunk cc of block b.
                cc 0..3 -> qT[cc], cc 4..7 -> kT[cc-4]."""
                bs = slice(512 * b, 512 * (b + 1))
                wsrc = wq_sb if cc < 4 else wk_sb
                co = 128 * (cc % 4)
                dst = qT_sb[cc] if cc < 4 else kT_sb[cc - 4]
                st = {}

                def mk(k0):
                    def step():
                        if k0 == 0:
                            st["ps"] = pw.tile([128, 512], F32, name="pw",
                                               tag="pw")
                        for k in (k0, k0 + 1):
                            nc.tensor.matmul(
                                st["ps"],
                                wsrc[:, 512 * k + co:512 * k + co + 128],
                                x_slice(k, 512 * b, 512 * (b + 1)),
                                start=(k == 0), stop=(k == KCH - 1),
                            )
                        if k0 == KCH - 2:
                            nc.vector.tensor_scalar_add(
                                out=dst[:, bs], in0=st["ps"],
                                scalar1=bqk_sb[:, cc:cc + 1])
                    return step
                return [mk(k0) for k0 in range(0, KCH, 2)]

            def v_steps(b, t2):
                """v projection for 128-token tile 4*b+t2 (augmented
                layout: ones col 0, dims 1:65)."""
                t = 4 * b + t2
                st = {}

                def mk(k0):
                    def step():
                        if k0 == 0:
                            st["ps"] = pw.tile([128, 512], F32, name="pw",
                                               tag="pw")
                        for k in (k0, k0 + 1):
                            nc.tensor.matmul(
                                st["ps"],
                                x_slice(k, 128 * t, 128 * (t + 1)),
                                wv_sb[:, 512 * k:512 * (k + 1)],
                                start=(k == 0), stop=(k == KCH - 1),
                            )
                        if k0 == KCH - 2:
                            va3 = vaug[t].rearrange("p (h c) -> p h c",
                                                    c=128)
                            nc.vector.tensor_add(
                                va3[:, :, 64:128],
                                st["ps"].rearrange("p (h d) -> p h d", d=HD),
                                bvb_sb.rearrange("p (h d) -> p h d", d=HD),
                            )
                    return step
                return [mk(k0) for k0 in range(0, KCH, 2)]

            def op_steps(q0, tq):
                """Output projection for 128-token tile 4*q0+tq."""
                t = 4 * q0 + tq
                st = {}

                def mk(nb, half):
                    def step():
                        if half == 0:
                            if nb == 0:
                                st["ob"] = outp.tile([128, D], F16, name="ob",
                                                     tag="ob")
                            st[nb] = pw.tile([128, 512], F32, name="pw",
                                             tag="pw")
                        for c in (2 * half, 2 * half + 1):
                            nc.tensor.matmul(
                                st[nb],
                                yT_sb[c][:, 128 * t:128 * (t + 1)],
                                wp_sb[:, 1024 * c + 512 * nb:
                                      1024 * c + 512 * (nb + 1)],
                                start=(c == 0), stop=(c == DL // 128 - 1),
                            )
                        if half == 1:
                            nc.vector.tensor_copy(
                                st["ob"][:, 512 * nb:512 * (nb + 1)], st[nb])
                            if nb == 1:
                                nc.sync.dma_start(
                                    out=out[128 * t:128 * (t + 1), :],
                                    in_=st["ob"])
                    return step
                return [mk(nb, half) for nb in range(2) for half in range(2)]

            EST_STEP = 460.0

            def enqueue(key, steps):
                for s in steps:
                    fillers.append((key, s))

            def pacer(delta):
                pace["d"] = min(pace["d"] + delta, 6000.0)
                while fillers and pace["d"] >= EST_STEP:
                    fillers.popleft()[1]()
                    pace["d"] -= EST_STEP
                pace["d"] = max(pace["d"], -2000.0)

            def drain(keys):
                # Selectively emit every queued step whose key matches,
                # preserving queue order among the rest.  A unit's steps
                # share one key and are contiguous, so units stay atomic.
                ks = set(keys)
                n = len(fillers)
                for _ in range(n):
                    key, fn = fillers.popleft()
                    if key in ks:
                        fn()
                    else:
                        fillers.append((key, fn))

            def drain_all():
                while fillers:
                    fillers.popleft()[1]()

            # ---------- attention chain (pend-depth 2) ----------
            def act_est(lo):
                n = 1024 if lo == 0 else 2 * (512 - lo)
                return (n + 352) / 1.2

            def s_est(lo):
                return 226.0 * (512 - lo) / 512 + 60

            def pv_est(lo):
                return 437.0 * (512 - lo) / 512 + 40

            def attn_chain(q0, c):
                """S/exp/PV over all key tiles of (q-block q0, head pair c),
                then normalize into yT."""
                qs_full = slice(512 * q0, 512 * (q0 + 1))
                ps_yA = psy.tile([128, 512], F32, name="psyA", tag="psyA")
                ps_yB = psy.tile([128, 512], F32, name="psyB", tag="psyB")
                ntile = 4 * q0 + 4
                pend = deque()

                def flush_pv(pes, pt, plo):
                    for p, psY in ((0, ps_yA), (1, ps_yB)):
                        h = 2 * c + p
                        nc.tensor.matmul(
                            psY[:, plo:512],
                            vaug[pt][:, 128 * h:128 * (h + 1)],
                            pes[:, 512 * p + plo:512 * (p + 1)],
                            start=(pt == 0), stop=(pt == ntile - 1))

                # 2-tile batches: [PV,PV] then [S,S] per iteration.  The PE
                # pays ~100ns on the first matmul after each row-pair <->
                # full-array config switch, so batching halves that cost.
                for i in range(ntile // 2):
                    delta = 0.0
                    # keep >=2 tiles of exp lead: only flush PV pairs that
                    # are two batches old (pend depth 4)
                    while len(pend) >= 4:
                        delta -= pv_est(pend[0][2])
                        flush_pv(*pend.popleft())
                    for t in (2 * i, 2 * i + 1):
                        m = t - 4 * q0
                        lo = 128 * m if m > 0 else 0
                        ks = slice(128 * t, 128 * (t + 1))
                        qs = slice(512 * q0 + lo, 512 * (q0 + 1))
                        ps_s = pss.tile([128, 1024], F32, name="psS",
                                        tag="psS")
                        nc.tensor.matmul(
                            ps_s[:, lo:512],
                            kT_sb[c][0:64, ks], qT_sb[c][0:64, qs],
                            start=True, stop=True,
                        )
                        nc.tensor.matmul(
                            ps_s[:, 512 + lo:1024],
                            kT_sb[c][64:128, ks], qT_sb[c][64:128, qs],
                            start=True, stop=True,
                        )
                        es = esb.tile([128, 1024], F16, name="es", tag="es")
                        if lo == 0:
                            nc.scalar.activation(
                                out=es[:, 0:1024], in_=ps_s[:, 0:1024],
                                func=mybir.ActivationFunctionType.Exp)
                        else:
                            nc.scalar.activation(
                                out=es[:, lo:512], in_=ps_s[:, lo:512],
                                func=mybir.ActivationFunctionType.Exp)
                            nc.scalar.activation(
                                out=es[:, 512 + lo:1024],
                                in_=ps_s[:, 512 + lo:1024],
                                func=mybir.ActivationFunctionType.Exp)
                        if m >= 0:
                            # causal mask: zero exp output where col < row
                            # in the 128x128 diagonal sub-block (both heads)
                            es3 = es.rearrange("p (h q) -> p h q", h=2)
                            nc.gpsimd.affine_select(
                                out=es3[:, :, lo:lo + 128],
                                in_=es3[:, :, lo:lo + 128],
                                pattern=[[0, 2], [1, 128]],
                                compare_op=mybir.AluOpType.is_ge,
                                fill=0.0,
                                base=0,
                                channel_multiplier=-1,
                            )
                        pend.append((es, t, lo))
                        delta += act_est(lo) - s_est(lo)
                    pacer(delta)
                while pend:
                    flush_pv(*pend.popleft())
                # normalize: PV row 0 is the denominator (ones-first vaug)
                for p, psY in ((0, ps_yA), (1, ps_yB)):
                    dn = small.tile([1, 512], F32, name="dn", tag="dn")
                    nc.vector.tensor_copy(dn, psY[0:1, :])
                    rc = small.tile([1, 512], F32, name="rc", tag="rc")
                    nc.vector.reciprocal_approx_fast(rc, dn)
                    rb = small.tile([64, 512], F32, name="rb", tag="rb")
                    nc.gpsimd.partition_broadcast(rb, rc)
                    nc.vector.tensor_mul(
                        yT_sb[c][64 * p:64 * (p + 1), qs_full],
                        psY[64:128, :],
                        rb,
                    )
                pace["d"] -= 2 * pv_est(pend[0][2] if pend else 0)

            # ---------- prologue: minimum to start chain (0, 0) ----------
            for s in qkv_steps(0, 0) + qkv_steps(0, 4):
                s()
            for t2 in range(2):
                for s in v_steps(0, t2):
                    s()
            for t2 in (2, 3):
                enqueue((0, -1), v_steps(0, t2))
            for cc in (1, 5, 2, 6, 3, 7):
                enqueue((0, cc % 4), qkv_steps(0, cc))

            # ---------- main block loop ----------
            for q0 in range(NQB):
                if q0 > 0:
                    for tq in range(4):
                        enqueue(None, op_steps(q0 - 1, tq))
                if q0 < NQB - 1:
                    for t2 in range(4):
                        enqueue((q0 + 1, -1), v_steps(q0 + 1, t2))
                    for cc in (0, 4, 1, 5, 2, 6, 3, 7):
                        enqueue((q0 + 1, cc % 4), qkv_steps(q0 + 1, cc))
                for c in range(4):
                    drain([(q0, c)] + ([(q0, -1)] if c == 0 else []))
                    attn_chain(q0, c)

            # ---------- epilogue ----------
            drain_all()
            for tq in range(4):
                for s in op_steps(NQB - 1, tq):
                    s()

    nc.finalize()
    return nc


def _enable_trace_hooks():
    """Inject antenv.axon_hooks + no-op artifact upload so that
    run_bass_kernel_spmd(trace=True) works under axon in this image."""
    import types
    import antenv

    if "antenv.axon_hooks" not in sys.modules:
        mod = types.ModuleType("antenv.axon_hooks")
        state = {"hook": None}
        mod.set_axon_ntff_profile_hook = lambda h: state.__setitem__("hook", h)
        mod.get_axon_ntff_profile_hook = lambda: state["hook"]
        sys.modules["antenv.axon_hooks"] = mod
        antenv.axon_hooks = mod
        from trn_agent_boot.trn_boot import _ntff_profile_via_ctypes

        mod.set_axon_ntff_profile_hook(
            _ntff_profile_via_ctypes("/opt/axon/libaxon_pjrt.so"))
    from concourse import bass_utils as bu

    bu.upload_artifacts = lambda tmpdir: str(tmpdir)


def kernel(x, w_attn, b_attn, w_proj, b_proj, _trace=False):
    x = np.asarray(x)
    w_attn = np.asarray(w_attn)
    b_attn = np.asarray(b_attn)
    w_proj = np.asarray(w_proj)
    b_proj = np.asarray(b_proj)

    if "nc" not in _cache:
        _cache["nc"] = _build()
    nc = _cache["nc"]

    scale = 1.0 / np.sqrt(HD)
    f16 = np.float16

    in_maps = []
    for core in range(8):
        b, hg = core // 2, core % 2
        qs = slice(hg * DL, (hg + 1) * DL)
        ks = slice(D + hg * DL, D + (hg + 1) * DL)
        vs = slice(2 * D + hg * DL, 2 * D + (hg + 1) * DL)
        bqk_host = np.concatenate(
            [b_attn[qs] * scale, b_attn[ks]]).astype(np.float32)
        in_maps.append({
            "xT": np.ascontiguousarray(x[b].T).astype(f16),
            "wq": np.ascontiguousarray(w_attn[:, qs] * scale).astype(f16),
            "wk": np.ascontiguousarray(w_attn[:, ks]).astype(f16),
            "bqk": np.ascontiguousarray(bqk_host.reshape(8, 128).T),
            "wv": np.ascontiguousarray(w_attn[:, vs]).astype(f16),
            "bv": np.ascontiguousarray(b_attn[vs][None, :]).astype(np.float32),
            "wp": np.ascontiguousarray(w_proj[hg * DL:(hg + 1) * DL, :]).astype(f16),
        })

    kwargs = {}
    if _trace:
        _enable_trace_hooks()
        kwargs = dict(trace=True, trace_cores=[0])
    res = run_bass_kernel_spmd(nc, in_maps, core_ids=list(range(8)), **kwargs)

    outp = np.empty((B, T, D), np.float32)
    for b in range(B):
        outp[b] = (np.asarray(res.results[2 * b]["out"], np.float32)
                   + np.asarray(res.results[2 * b + 1]["out"], np.float32))
    outp += b_proj.astype(np.float32)

    if _trace:
        print(f"HW exec time: {res.exec_time_ns} ns")
    return outp


# revision 27
# speedup vs baseline: 1.4402x; 1.1985x over previous
"""Causal self-attention (B=4, T=2048, D=1024, H=16) on 8 trn2 NeuronCores.

Sharding: Megatron-style tensor parallel over heads (TP=2) x data parallel
over batch (DP=4).  Core c handles batch c//2 and head-group c%2 (8 heads).
Each core computes its QKV projection slice, causal attention for its 8
heads, and a partial output projection; the host sums the two TP partials
per batch and adds b_proj.

v9 schedule: natural chain order + credit-paced micro-fillers.
  - Chains run in natural (q0, c) order; no pulled-forward spills.  The
    scalar engine's exp stream (1.1us per full key tile) sets the cadence
    inside attention; a credit pacer weaves 2-matmul filler micro-steps
    (QKV projection of the next block, output projection of the previous)
    into the PE slack of each tile.
  - Score matmuls contract over the 64-deep head dim; the two heads of a
    pair run as two concurrent row-group matmuls (PE rows 0-63 / 64-127).
  - Scores are issued with pend-depth 2 (S(t) runs two tiles ahead of
    PV(t)) so weight loads hide under neighbouring matmuls.
  - Diagonal-tile exps and masks use one 3D-AP instruction covering both
    heads (halves scalar-engine instruction count there).
  - vaug puts the softmax-denominator ones column FIRST, so the PV output
    row 0 is the denominator at PSUM partition 0: the reciprocal reads it
    directly and the old staging copy disappears.
All matmuls run in fp16 (fp32 PSUM accumulation); softmax in fp32 on the
scalar engine (exp) / DVE (reciprocal).  Output partials are written fp16
(host accumulates in fp32).
"""
import sys

sys.path.insert(0, "/opt/trn_rl_repo")

from collections import deque

import numpy as np

import concourse.bass as bass
import concourse.tile as tile
from concourse import bacc, mybir
from concourse.bass_utils import run_bass_kernel_spmd

B, T, D, H = 4, 2048, 1024, 16
HD = 64            # head dim
HL = 8             # heads per core (TP=2)
DL = HL * HD       # 512 local qkv width
KCH = D // 128     # 8 contraction chunks for QKV
NQB = T // 512     # 4 query blocks of 512
F16 = mybir.dt.float16
F32 = mybir.dt.float32
NEG = -1.0e30

_cache = {}


def _build():
    nc = bacc.Bacc("TRN2", target_bir_lowering=False, num_devices=8)

    xT = nc.dram_tensor("xT", [D, T], F16, kind="ExternalInput")
    wq = nc.dram_tensor("wq", [D, DL], F16, kind="ExternalInput")
    wk = nc.dram_tensor("wk", [D, DL], F16, kind="ExternalInput")
    bqk = nc.dram_tensor("bqk", [128, 2 * DL // 128], F32, kind="ExternalInput")
    wv = nc.dram_tensor("wv", [D, DL], F16, kind="ExternalInput")
    bv = nc.dram_tensor("bv", [1, DL], F32, kind="ExternalInput")
    wp = nc.dram_tensor("wp", [DL, D], F16, kind="ExternalInput")
    out = nc.dram_tensor("out", [T, D], F16, kind="ExternalOutput")

    with tile.TileContext(nc) as tc:
        with (
            tc.tile_pool(name="const", bufs=1) as const,
            tc.tile_pool(name="acts", bufs=1) as acts,
            tc.tile_pool(name="esb", bufs=6) as esb,
            tc.tile_pool(name="small", bufs=3) as small,
            tc.tile_pool(name="outp", bufs=3) as outp,
            tc.tile_pool(name="pss", bufs=2, space="PSUM") as pss,
            tc.tile_pool(name="psy", bufs=1, space="PSUM") as psy,
            tc.tile_pool(name="pw", bufs=2, space="PSUM") as pw,
        ):
            # ---- PE warm-up: keep the HAM busy while inputs stream in ----
            zw = const.tile([128, 128], F16, name="zw", tag="zw")
            nc.gpsimd.memset(zw, 0.0)
            for i in range(48):
                psz = pw.tile([128, 512], F32, name="pw", tag="pw")
                nc.tensor.matmul(psz[:, 0:128], zw, zw, start=True, stop=True)

            # ---- inputs: per-chunk DMAs so compute starts early ----
            xb0_sb = const.tile([128, KCH * 512], F16, name="xb0", tag="xb0")
            xrest_sb = const.tile([128, KCH * 1536], F16, name="xrest",
                                  tag="xrest")
            wq_sb = const.tile([128, KCH * 512], F16, name="wq", tag="wq")
            wk_sb = const.tile([128, KCH * 512], F16, name="wk", tag="wk")
            wv_sb = const.tile([128, KCH * 512], F16, name="wv", tag="wv")
            wp_sb = const.tile([128, 4 * D], F16, name="wp", tag="wp")
            bqk_sb = const.tile([128, 2 * DL // 128], F32)
            bv_sb = const.tile([1, DL], F32)
            x3 = xT.rearrange("(k p) t -> p k t", p=128)
            q3 = wq.rearrange("(k p) n -> p k n", p=128)
            k3 = wk.rearrange("(k p) n -> p k n", p=128)
            v3 = wv.rearrange("(k p) n -> p k n", p=128)
            p3 = wp.rearrange("(c p) n -> p c n", p=128)
            # descriptor issue costs ~650ns each on the Sync engine, so use
            # few big DMAs (runtime splits each across the 16 queues),
            # critical tensors first
            wq4 = wq_sb.rearrange("p (k n) -> p k n", k=KCH)
            xb4 = xb0_sb.rearrange("p (k t) -> p k t", k=KCH)
            h = KCH // 2
            nc.sync.dma_start(out=wq4[:, 0:h], in_=q3[:, 0:h, :])
            nc.sync.dma_start(out=xb4[:, 0:h], in_=x3[:, 0:h, 0:512])
            nc.sync.dma_start(out=wq4[:, h:KCH], in_=q3[:, h:KCH, :])
            nc.sync.dma_start(out=xb4[:, h:KCH], in_=x3[:, h:KCH, 0:512])
            nc.sync.dma_start(
                out=wk_sb.rearrange("p (k n) -> p k n", k=KCH),
                in_=k3[:, :, :])
            nc.sync.dma_start(out=bqk_sb, in_=bqk[:, :])
            nc.sync.dma_start(out=bv_sb, in_=bv[:, :])
            nc.sync.dma_start(
                out=wv_sb.rearrange("p (k n) -> p k n", k=KCH),
                in_=v3[:, :, :])
            nc.sync.dma_start(
                out=xrest_sb.rearrange("p (k t) -> p k t", k=KCH),
                in_=x3[:, :, 512:T])
            nc.sync.dma_start(
                out=wp_sb.rearrange("p (c n) -> p c n", c=4),
                in_=p3[:, :, :])
            bvb_sb = const.tile([128, DL], F32)
            nc.gpsimd.partition_broadcast(bvb_sb, bv_sb)

            def x_slice(k, c0, c1):
                """xT chunk k, token-columns [c0:c1)."""
                if c1 <= 512:
                    return xb0_sb[:, 512 * k + c0:512 * k + c1]
                return xrest_sb[:, 1536 * k + c0 - 512:1536 * k + c1 - 512]

            # ---- persistent activations ----
            # qT/kT tile c: partitions 0:64 = head 2c dims, 64:128 = head
            # 2c+1 dims; free dim = T.  yT same channel layout.
            qT_sb = [acts.tile([128, T], F16, name=f"qT{c}", tag=f"qT{c}")
                     for c in range(4)]
            kT_sb = [acts.tile([128, T], F16, name=f"kT{c}", tag=f"kT{c}")
                     for c in range(4)]
            # vaug: per head 128 cols: col 0 = ones (softmax denominator),
            # 1:64 zero pad, 64:128 = v dims.  PV output row 0 is then the
            # denominator at PSUM partition 0 (direct reciprocal read) and
            # the y dims sit 32-aligned at partitions 64:128.
            vaug = [acts.tile([128, HL * 128], F16, name=f"va{t}",
                              tag=f"va{t}") for t in range(T // 128)]
            for t in range(T // 128):
                va3 = vaug[t].rearrange("p (h c) -> p h c", c=128)
                nc.gpsimd.memset(va3[:, :, 0:64], 0.0)
                nc.gpsimd.memset(va3[:, :, 0], 1.0)
            yT_sb = [acts.tile([128, T], F16, name=f"yT{c}", tag=f"yT{c}")
                     for c in range(4)]

            # ---------- filler micro-steps ----------
            # Each filler unit is broken into ~2-matmul closures enqueued
            # contiguously (FIFO drain keeps at most one unit open per pw
            # buffer).  Key (q0, c) marks steps that must drain before
            # chain (q0, c) starts; key None = deadline-free (op units).
            fillers = deque()
            pace = {"d": 0.0}

            def qkv_steps(b, cc):
                """Projection chain for output chunk cc of block b.
                cc 0..3 -> qT[cc], cc 4..7 -> kT[cc-4]."""
                bs = slice(512 * b, 512 * (b + 1))
                wsrc = wq_sb if cc < 4 else wk_sb
                co = 128 * (cc % 4)
                dst = qT_sb[cc] if cc < 4 else kT_sb[cc - 4]
                st = {}

                def mk(k0):
                    def step():
                        if k0 == 0:
                            st["ps"] = pw.tile([128, 512], F32, name="pw",
                                               tag="pw")
                        for k in (k0, k0 + 1):
                            nc.tensor.matmul(
                                st["ps"],
                                wsrc[:, 512 * k + co:512 * k + co + 128],
                                x_slice(k, 512 * b, 512 * (b + 1)),
                                start=(k == 0), stop=(k == KCH - 1),
                            )
                        if k0 == KCH - 2:
                            nc.vector.tensor_scalar_add(
                                out=dst[:, bs], in0=st["ps"],
                                scalar1=bqk_sb[:, cc:cc + 1])
                    return step
                return [mk(k0) for k0 in range(0, KCH, 2)]

            def v_steps(b, t2):
                """v projection for 128-token tile 4*b+t2 (augmented
                layout: ones col 0, dims 1:65)."""
                t = 4 * b + t2
                st = {}

                def mk(k0):
                    def step():
                        if k0 == 0:
                            st["ps"] = pw.tile([128, 512], F32, name="pw",
                                               tag="pw")
                        for k in (k0, k0 + 1):
                            nc.tensor.matmul(
                                st["ps"],
                                x_slice(k, 128 * t, 128 * (t + 1)),
                                wv_sb[:, 512 * k:512 * (k + 1)],
                                start=(k == 0), stop=(k == KCH - 1),
                            )
                        if k0 == KCH - 2:
                            va3 = vaug[t].rearrange("p (h c) -> p h c",
                                                    c=128)
                            nc.vector.tensor_add(
                                va3[:, :, 64:128],
                                st["ps"].rearrange("p (h d) -> p h d", d=HD),
                                bvb_sb.rearrange("p (h d) -> p h d", d=HD),
                            )
                    return step
                return [mk(k0) for k0 in range(0, KCH, 2)]

            def op_steps(q0, tq):
                """Output projection for 128-token tile 4*q0+tq."""
                t = 4 * q0 + tq
                st = {}

                def mk(nb, half):
                    def step():
                        if half == 0:
                            if nb == 0:
                                st["ob"] = outp.tile([128, D], F16, name="ob",
                                                     tag="ob")
                            st[nb] = pw.tile([128, 512], F32, name="pw",
                                             tag="pw")
                        for c in (2 * half, 2 * half + 1):
                            nc.tensor.matmul(
                                st[nb],
                                yT_sb[c][:, 128 * t:128 * (t + 1)],
                                wp_sb[:, 1024 * c + 512 * nb:
                                      1024 * c + 512 * (nb + 1)],
                                start=(c == 0), stop=(c == DL // 128 - 1),
                            )
                        if half == 1:
                            nc.vector.tensor_copy(
                                st["ob"][:, 512 * nb:512 * (nb + 1)], st[nb])
                            if nb == 1:
                                nc.sync.dma_start(
                                    out=out[128 * t:128 * (t + 1), :],
                                    in_=st["ob"])
                    return step
                return [mk(nb, half) for nb in range(2) for half in range(2)]

            EST_STEP = 460.0

            def enqueue(key, steps):
                for s in steps:
                    fillers.append((key, s))

            def pacer(delta):
                pace["d"] = min(pace["d"] + delta, 6000.0)
                while fillers and pace["d"] >= EST_STEP:
                    fillers.popleft()[1]()
                    pace["d"] -= EST_STEP
                pace["d"] = max(pace["d"], -2000.0)

            def drain(keys):
                # Selectively emit every queued step whose key matches,
                # preserving queue order among the rest.  A unit's steps
                # share one key and are contiguous, so units stay atomic.
                ks = set(keys)
                n = len(fillers)
                for _ in range(n):
                    key, fn = fillers.popleft()
                    if key in ks:
                        fn()
                    else:
                        fillers.append((key, fn))

            def drain_all():
                while fillers:
                    fillers.popleft()[1]()

            # ---------- attention chain (pend-depth 2) ----------
            def act_est(lo):
                n = 1024 if lo == 0 else 2 * (512 - lo)
                return (n + 352) / 1.2

            def s_est(lo):
                return 226.0 * (512 - lo) / 512 + 60

            def pv_est(lo):
                return 437.0 * (512 - lo) / 512 + 40

            def attn_chain(q0, c, pre_norm=None):
                """S/exp/PV over all key tiles of (q-block q0, head pair c),
                then normalize into yT."""
                qs_full = slice(512 * q0, 512 * (q0 + 1))
                ps_yA = psy.tile([128, 512], F32, name="psyA", tag="psyA")
                ps_yB = psy.tile([128, 512], F32, name="psyB", tag="psyB")
                ntile = 4 * q0 + 4
                pend = deque()

                def flush_pv(pes, pt, plo):
                    for p, psY in ((0, ps_yA), (1, ps_yB)):
                        h = 2 * c + p
                        nc.tensor.matmul(
                            psY[:, plo:512],
                            vaug[pt][:, 128 * h:128 * (h + 1)],
                            pes[:, 512 * p + plo:512 * (p + 1)],
                            start=(pt == 0), stop=(pt == ntile - 1))

                # 2-tile batches: [PV,PV] then [S,S] per iteration.  The PE
                # pays ~100ns on the first matmul after each row-pair <->
                # full-array config switch, so batching halves that cost.
                for i in range(ntile // 2):
                    delta = 0.0
                    # keep >=2 tiles of exp lead: only flush PV pairs that
                    # are two batches old (pend depth 4)
                    while len(pend) >= 4:
                        delta -= pv_est(pend[0][2])
                        flush_pv(*pend.popleft())
                    for t in (2 * i, 2 * i + 1):
                        m = t - 4 * q0
                        lo = 128 * m if m > 0 else 0
                        ks = slice(128 * t, 128 * (t + 1))
                        qs = slice(512 * q0 + lo, 512 * (q0 + 1))
                        ps_s = pss.tile([128, 1024], F32, name="psS",
                                        tag="psS")
                        nc.tensor.matmul(
                            ps_s[:, lo:512],
                            kT_sb[c][0:64, ks], qT_sb[c][0:64, qs],
                            start=True, stop=True,
                        )
                        nc.tensor.matmul(
                            ps_s[:, 512 + lo:1024],
                            kT_sb[c][64:128, ks], qT_sb[c][64:128, qs],
                            start=True, stop=True,
                        )
                        es = esb.tile([128, 1024], F16, name="es", tag="es")
                        if lo == 0:
                            nc.scalar.activation(
                                out=es[:, 0:1024], in_=ps_s[:, 0:1024],
                                func=mybir.ActivationFunctionType.Exp)
                        else:
                            nc.scalar.activation(
                                out=es[:, lo:512], in_=ps_s[:, lo:512],
                                func=mybir.ActivationFunctionType.Exp)
                            nc.scalar.activation(
                                out=es[:, 512 + lo:1024],
                                in_=ps_s[:, 512 + lo:1024],
                                func=mybir.ActivationFunctionType.Exp)
                        if m >= 0:
                            # causal mask: zero exp output where col < row
                            # in the 128x128 diagonal sub-block (both heads)
                            es3 = es.rearrange("p (h q) -> p h q", h=2)
                            nc.gpsimd.affine_select(
                                out=es3[:, :, lo:lo + 128],
                                in_=es3[:, :, lo:lo + 128],
                                pattern=[[0, 2], [1, 128]],
                                compare_op=mybir.AluOpType.is_ge,
                                fill=0.0,
                                base=0,
                                channel_multiplier=-1,
                            )
                        pend.append((es, t, lo))
                        delta += act_est(lo) - s_est(lo)
                    pacer(delta)
                while pend:
                    flush_pv(*pend.popleft())
                if pre_norm is not None:
                    # emit the next chain's queued projection steps before
                    # our normalize so their DVE bias-adds aren't stuck
                    # behind it in the DVE queue
                    pre_norm()
                # normalize: PV row 0 is the denominator (ones-first vaug)
                for p, psY in ((0, ps_yA), (1, ps_yB)):
                    dn = small.tile([1, 512], F32, name="dn", tag="dn")
                    nc.vector.tensor_copy(dn, psY[0:1, :])
                    rc = small.tile([1, 512], F32, name="rc", tag="rc")
                    nc.vector.reciprocal_approx_fast(rc, dn)
                    rb = small.tile([64, 512], F32, name="rb", tag="rb")
                    nc.gpsimd.partition_broadcast(rb, rc)
                    nc.vector.tensor_mul(
                        yT_sb[c][64 * p:64 * (p + 1), qs_full],
                        psY[64:128, :],
                        rb,
                    )
                pace["d"] -= 2 * pv_est(pend[0][2] if pend else 0)

            # ---------- prologue: minimum to start chain (0, 0) ----------
            for s in qkv_steps(0, 0) + qkv_steps(0, 4):
                s()
            for t2 in range(2):
                for s in v_steps(0, t2):
                    s()
            for t2 in (2, 3):
                enqueue((0, -1), v_steps(0, t2))
            for cc in (1, 5, 2, 6, 3, 7):
                enqueue((0, cc % 4), qkv_steps(0, cc))

            # ---------- main block loop ----------
            for q0 in range(NQB):
                if q0 > 0:
                    for tq in range(4):
                        enqueue(None, op_steps(q0 - 1, tq))
                if q0 < NQB - 1:
                    for t2 in range(4):
                        enqueue((q0 + 1, -1), v_steps(q0 + 1, t2))
                    for cc in (0, 4, 1, 5, 2, 6, 3, 7):
                        enqueue((q0 + 1, cc % 4), qkv_steps(q0 + 1, cc))
                for c in range(4):
                    if c < 3:
                        nxt = [(q0, c + 1)]
                    elif q0 < NQB - 1:
                        nxt = [(q0 + 1, 0), (q0 + 1, -1)]
                    else:
                        nxt = None
                    drain([(q0, c)] + ([(q0, -1)] if c == 0 else []))
                    attn_chain(q0, c,
                               pre_norm=(lambda keys=nxt: drain(keys))
                               if nxt else None)

            # ---------- epilogue ----------
            drain_all()
            for tq in range(4):
                for s in op_steps(NQB - 1, tq):
                    s()

    nc.finalize()
    return nc


def _enable_trace_hooks():
    """Inject antenv.axon_hooks + no-op artifact upload so that
    run_bass_kernel_spmd(trace=True) works under axon in this image."""
    import types
    import antenv

    if "antenv.axon_hooks" not in sys.modules:
        mod = types.ModuleType("antenv.axon_hooks")
        state = {"hook": None}
        mod.set_axon_ntff_profile_hook = lambda h: state.__setitem__("hook", h)
        mod.get_axon_ntff_profile_hook = lambda: state["hook"]
        sys.modules["antenv.axon_hooks"] = mod
        antenv.axon_hooks = mod
        from trn_agent_boot.trn_boot import _ntff_profile_via_ctypes

        mod.set_axon_ntff_profile_hook(
            _ntff_profile_via_ctypes("/opt/axon/libaxon_pjrt.so"))
    from concourse import bass_utils as bu

    bu.upload_artifacts = lambda tmpdir: str(tmpdir)


def kernel(x, w_attn, b_attn, w_proj, b_proj, _trace=False):
    x = np.asarray(x)
    w_attn = np.asarray(w_attn)
    b_attn = np.asarray(b_attn)
    w_proj = np.asarray(w_proj)
    b_proj = np.asarray(b_proj)

    if "nc" not in _cache:
        _cache["nc"] = _build()
    nc = _cache["nc"]

    scale = 1.0 / np.sqrt(HD)
    f16 = np.float16

    in_maps = []
    for core in range(8):
        b, hg = core // 2, core % 2
        qs = slice(hg * DL, (hg + 1) * DL)
        ks = slice(D + hg * DL, D + (hg + 1) * DL)
        vs = slice(2 * D + hg * DL, 2 * D + (hg + 1) * DL)
        bqk_host = np.concatenate(
            [b_attn[qs] * scale, b_attn[ks]]).astype(np.float32)
        in_maps.append({
            "xT": np.ascontiguousarray(x[b].T).astype(f16),
            "wq": np.ascontiguousarray(w_attn[:, qs] * scale).astype(f16),
            "wk": np.ascontiguousarray(w_attn[:, ks]).astype(f16),
            "bqk": np.ascontiguousarray(bqk_host.reshape(8, 128).T),
            "wv": np.ascontiguousarray(w_attn[:, vs]).astype(f16),
            "bv": np.ascontiguousarray(b_attn[vs][None, :]).astype(np.float32),
            "wp": np.ascontiguousarray(w_proj[hg * DL:(hg + 1) * DL, :]).astype(f16),
        })

    kwargs = {}
    if _trace:
        _enable_trace_hooks()
        kwargs = dict(trace=True, trace_cores=[0])
    res = run_bass_kernel_spmd(nc, in_maps, core_ids=list(range(8)), **kwargs)

    outp = np.empty((B, T, D), np.float32)
    for b in range(B):
        outp[b] = (np.asarray(res.results[2 * b]["out"], np.float32)
                   + np.asarray(res.results[2 * b + 1]["out"], np.float32))
    outp += b_proj.astype(np.float32)

    if _trace:
        print(f"HW exec time: {res.exec_time_ns} ns")
    return outp
